# revision 1
# baseline (speedup 1.0000x reference)
"""BN(train) -> binarize -> conv1d(K=7,pad=3) -> alpha-scale -> maxpool2 on 8 trn2 cores.

Data-parallel over batch N: each core owns 8 samples. BN statistics are
computed per-core with bn_stats/bn_aggr and combined with a tiny [64,2]
AllReduce of (mean, E[x^2]) partials. alpha is folded into the conv
weights on the host (alpha > 0 commutes with maxpool).

Per-core layout: each sample is held in SBUF as [128, 4102] fp32 where
partitions 0-63 = channels for L 0..4095 (+halo) and partitions 64-127 =
channels for L 4096..8191 (+halo). Binarize is one ScalarE Sign op into
bf16. The conv runs as 7 taps x 2 concurrent K=64 matmuls (row groups 0
and 64), accumulating fp32 in PSUM; the maxpool is a strided pairwise
max on VectorE.
"""

import numpy as np
from contextlib import ExitStack

N, CIN, L = 64, 64, 8192
COUT, K = 128, 7
NCORES = 8
NS = N // NCORES          # samples per core
HALF = L // 2             # 4096
PAD = K // 2              # 3
WCOLS = HALF + 2 * PAD    # 4102
T = 512                   # conv output cols per matmul tile
NT = HALF // T            # 8 tiles per half
TP = T // 2               # pooled cols per tile
EPS = 1e-5

_CACHE = {}


def _build():
    import concourse.bass as bass
    import concourse.tile as tile
    from concourse import mybir

    f32 = mybir.dt.float32
    bf16 = mybir.dt.bfloat16
    Alu = mybir.AluOpType
    Act = mybir.ActivationFunctionType

    nc = bass.Bass()
    # I is pre-transposed on the host to [NS, 128, HALF]: row h*64+c holds
    # I[n, c, h*HALF : (h+1)*HALF]. A 2D regular 128-row DRAM source is the
    # only load shape that reaches fabric-rate (~430 GB/s) on the HWDGE path.
    I_h = nc.declare_dram_parameter("I", [NS, 128, HALF], f32, isOutput=False)
    Wt_h = nc.declare_dram_parameter("Wt", [K, CIN, COUT], f32, isOutput=False)
    g_h = nc.declare_dram_parameter("gamma", [CIN], f32, isOutput=False)
    be_h = nc.declare_dram_parameter("beta", [CIN], f32, isOutput=False)
    out_h = nc.declare_dram_parameter("out", [NS, COUT, HALF], f32, isOutput=True)

    with ExitStack() as ctx:
        tc = ctx.enter_context(tile.TileContext(nc))
        singles = ctx.enter_context(tc.tile_pool(name="singles", bufs=1))
        ibufs = ctx.enter_context(tc.tile_pool(name="ibufs", bufs=NS))
        xbs = ctx.enter_context(tc.tile_pool(name="xbs", bufs=3))
        psums = ctx.enter_context(tc.tile_pool(name="psums", bufs=3, space="PSUM"))
        psum_warm = ctx.enter_context(
            tc.tile_pool(name="psum_warm", bufs=1, space="PSUM")
        )
        stages = ctx.enter_context(tc.tile_pool(name="stages", bufs=3))
        dram = ctx.enter_context(tc.tile_pool(name="dram", bufs=1, space="DRAM"))

        # --- weights: [128, K, COUT] bf16, channels duplicated on both
        # partition halves so row-group-0 and row-group-64 matmuls can run
        # concurrently.
        w_f32 = singles.tile([128, K, COUT], f32)
        w_src = Wt_h[:].rearrange("k c o -> c k o")
        nc.sync.dma_start(out=w_f32[0:CIN, :, :], in_=w_src)
        nc.sync.dma_start(out=w_f32[CIN:128, :, :], in_=w_src)
        wsb = singles.tile([128, K, COUT], bf16)
        nc.vector.tensor_copy(out=wsb[:], in_=w_f32[:])

        gam = singles.tile([CIN, 1], f32)
        bet = singles.tile([CIN, 1], f32)
        nc.sync.dma_start(out=gam[:], in_=g_h[:].rearrange("(c o) -> c o", o=1))
        nc.sync.dma_start(out=bet[:], in_=be_h[:].rearrange("(c o) -> c o", o=1))
        eps_t = singles.tile([CIN, 1], f32)
        nc.vector.memset(eps_t[:], EPS)

        # throwaway AllReduce issued with no deps: pays the ncfw wake /
        # channel-setup cost concurrently with the input DMA phase so the
        # real stats AllReduce hits a warm path
        warm_src = singles.tile([CIN, 2], f32)
        nc.vector.memset(warm_src[:], 0.0)
        war_in = dram.tile([CIN, 2], f32)
        war_out = dram.tile([CIN, 2], f32, addr_space="Shared")
        nc.gpsimd.dma_start(out=war_in[:], in_=warm_src[:])
        nc.gpsimd.collective_compute(
            "AllReduce",
            Alu.add,
            replica_groups=[list(range(NCORES))],
            ins=[war_in[:]],
            outs=[war_out[:]],
        )

        # --- phase 1: load + per-partition stats -------------------------
        stats = singles.tile([128, NS * NT, 6], f32)
        ibs = []
        for n in range(NS):
            ib = ibufs.tile([128, WCOLS], f32, name=f"ib{n}", tag="ib")
            ibs.append(ib)
            # zero the halo columns that fall off the sequence ends
            nc.vector.memset(ib[0:CIN, 0:PAD], 0.0)
            nc.vector.memset(ib[CIN:128, WCOLS - PAD : WCOLS], 0.0)
            eng = nc.sync if n % 2 == 0 else nc.scalar
            eng.dma_start(out=ib[:, PAD : PAD + HALF], in_=I_h[n])
            # halo edges: lo needs positions 4096..4098 (hi rows, cols 0..2),
            # hi needs positions 4093..4095 (lo rows, last 3 cols)
            nc.gpsimd.dma_start(
                out=ib[0:CIN, WCOLS - PAD : WCOLS], in_=I_h[n, CIN:128, 0:PAD]
            )
            nc.gpsimd.dma_start(
                out=ib[CIN:128, 0:PAD], in_=I_h[n, 0:CIN, HALF - PAD : HALF]
            )
            for c in range(NT):
                nc.vector.bn_stats(
                    out=stats[:, n * NT + c, :],
                    in_=ib[:, PAD + c * T : PAD + (c + 1) * T],
                )

        # --- fold stats: per-partition (mean, var) -> global (s, b) ------
        mv = singles.tile([128, 2], f32)
        nc.vector.bn_aggr(out=mv[:], in_=stats[:])
        # ms = (mean, E[x^2]) per partition
        ms = singles.tile([128, 2], f32)
        nc.vector.tensor_copy(out=ms[:, 0:1], in_=mv[:, 0:1])
        nc.vector.tensor_tensor(
            out=ms[:, 1:2], in0=mv[:, 0:1], in1=mv[:, 0:1], op=Alu.mult
        )
        nc.vector.tensor_tensor(
            out=ms[:, 1:2], in0=ms[:, 1:2], in1=mv[:, 1:2], op=Alu.add
        )
        upper = singles.tile([CIN, 2], f32)
        nc.sync.dma_start(out=upper[:], in_=ms[CIN:128, :])
        msum = singles.tile([CIN, 2], f32)
        nc.vector.tensor_tensor(
            out=msum[:], in0=ms[0:CIN, :], in1=upper[:], op=Alu.add
        )

        ar_in = dram.tile([CIN, 2], f32)
        ar_out = dram.tile([CIN, 2], f32, addr_space="Shared")
        nc.gpsimd.dma_start(out=ar_in[:], in_=msum[:])
        nc.gpsimd.collective_compute(
            "AllReduce",
            Alu.add,
            replica_groups=[list(range(NCORES))],
            ins=[ar_in[:]],
            outs=[ar_out[:]],
        )
        arsb = singles.tile([CIN, 2], f32)
        nc.gpsimd.dma_start(out=arsb[:], in_=ar_out[:])

        # keep the PE busy across the AllReduce wait so the HAM clock gate
        # stays open when the real conv matmuls arrive: a junk matmul chain
        # gated on msum (so it runs inside the wait window, not earlier)
        warm_rhs = singles.tile([CIN, T], bf16)
        nc.vector.memset(warm_rhs[:], 1.0)
        nc.vector.tensor_copy(out=warm_rhs[:, 0:2], in_=msum[:])
        ps_warm = psum_warm.tile([COUT, T], f32, name="ps_warm")
        for w in range(16):
            nc.tensor.matmul(
                ps_warm[:],
                wsb[0:CIN, 0, :],
                warm_rhs[:],
                start=(w == 0),
                stop=(w == 15),
            )

        # global mean / var per channel (16 partition-partials, equal counts)
        q = singles.tile([CIN, 2], f32)
        tmp = singles.tile([CIN, 1], f32)
        var = singles.tile([CIN, 1], f32)
        rstd = singles.tile([CIN, 1], f32)
        nc.vector.tensor_scalar_mul(q[:], arsb[:], 1.0 / 16.0)
        nc.vector.tensor_tensor(
            out=tmp[:], in0=q[:, 0:1], in1=q[:, 0:1], op=Alu.mult
        )
        nc.vector.tensor_tensor(
            out=var[:], in0=q[:, 1:2], in1=tmp[:], op=Alu.subtract
        )
        # rstd = 1/sqrt(var + eps)
        nc.scalar.activation(
            out=rstd[:], in_=var[:], func=Act.Sqrt, bias=eps_t[:], scale=1.0
        )
        nc.vector.reciprocal(out=rstd[:], in_=rstd[:])

        # sb2 = (s, b) = (gamma*rstd, beta - mean*s) on both partition halves
        sb2 = singles.tile([128, 2], f32)
        nc.vector.tensor_tensor(
            out=sb2[0:CIN, 0:1], in0=gam[:], in1=rstd[:], op=Alu.mult
        )
        nc.vector.tensor_tensor(
            out=tmp[:], in0=q[:, 0:1], in1=sb2[0:CIN, 0:1], op=Alu.mult
        )
        nc.vector.tensor_tensor(
            out=sb2[0:CIN, 1:2], in0=bet[:], in1=tmp[:], op=Alu.subtract
        )
        nc.sync.dma_start(out=sb2[CIN:128, :], in_=sb2[0:CIN, :])

        # --- phase 2: binarize + conv + pool -----------------------------
        for n in range(NS):
            ib = ibs[n]
            xb = xbs.tile([128, WCOLS], bf16, name="xb")
            nc.scalar.activation(
                out=xb[:],
                in_=ib[:],
                func=Act.Sign,
                bias=sb2[:, 1:2],
                scale=sb2[:, 0:1],
            )
            # conv pads with zeros (binarize happens before padding)
            nc.vector.memset(xb[0:CIN, 0:PAD], 0.0)
            nc.vector.memset(xb[CIN:128, WCOLS - PAD : WCOLS], 0.0)

            for t in range(NT):
                ps_lo = psums.tile([COUT, T], f32, name="ps_lo")
                ps_hi = psums.tile([COUT, T], f32, name="ps_hi")
                for k in range(K):
                    a = t * T + k
                    nc.tensor.matmul(
                        ps_lo[:],
                        wsb[0:CIN, k, :],
                        xb[0:CIN, a : a + T],
                        start=(k == 0),
                        stop=(k == K - 1),
                    )
                    nc.tensor.matmul(
                        ps_hi[:],
                        wsb[CIN:128, k, :],
                        xb[CIN:128, a : a + T],
                        start=(k == 0),
                        stop=(k == K - 1),
                    )
                st_lo = stages.tile([COUT, TP], f32, name="st_lo")
                st_hi = stages.tile([COUT, TP], f32, name="st_hi")
                lo_v = ps_lo[:].rearrange("p (n two) -> p n two", two=2)
                hi_v = ps_hi[:].rearrange("p (n two) -> p n two", two=2)
                # pairwise max along the innermost dim: one PSUM read port
                nc.vector.tensor_reduce(
                    out=st_lo[:], in_=lo_v, axis=mybir.AxisListType.X, op=Alu.max
                )
                nc.vector.tensor_reduce(
                    out=st_hi[:], in_=hi_v, axis=mybir.AxisListType.X, op=Alu.max
                )
                nc.sync.dma_start(
                    out=out_h[n, :, t * TP : (t + 1) * TP], in_=st_lo[:]
                )
                nc.scalar.dma_start(
                    out=out_h[n, :, HALF // 2 + t * TP : HALF // 2 + (t + 1) * TP],
                    in_=st_hi[:],
                )

    return nc


def _split_multi_waits(nc):
    """walrus codegen only supports one sync-wait command per instruction;
    the TileContext exit drain carries several. Split the extras onto NOPs
    inserted immediately before the offending instruction."""
    import bass_rust
    from concourse import mybir

    for f in nc.m.functions:
        for bb in f.blocks:
            idx = 0
            while idx < len(bb.instructions):
                ins = bb.instructions[idx]
                si = ins.sync_info
                if si is not None and si.on_wait and len(si.on_wait) > 1:
                    waits = list(si.on_wait)
                    keep, rest = waits[-1], waits[:-1]
                    ins.sync_info = bass_rust.SyncInfo(
                        on_wait=[keep], on_update=list(si.on_update or [])
                    )
                    new_insts = []
                    for w in rest:
                        nop = mybir.InstNoOp(
                            name=nc.get_next_instruction_name(), ins=[], outs=[]
                        )
                        nop.engine = ins.engine
                        nop.sync_info = bass_rust.SyncInfo(on_wait=[w], on_update=[])
                        new_insts.append(nop)
                    for j, nop in enumerate(new_insts):
                        bb.instructions.insert(idx + j, nop)
                    idx += len(new_insts)
                idx += 1


def _get_nc(split=True):
    key = ("nc", split)
    if key not in _CACHE:
        nc = _build()
        if split:
            _split_multi_waits(nc)
        _CACHE[key] = nc
    return _CACHE[key]


def _make_in_maps(I, gamma, beta, W, alpha):
    I = np.asarray(I, dtype=np.float32)
    gamma = np.ascontiguousarray(np.asarray(gamma, dtype=np.float32))
    beta = np.ascontiguousarray(np.asarray(beta, dtype=np.float32))
    W = np.asarray(W, dtype=np.float32)
    alpha = np.asarray(alpha, dtype=np.float32)

    # stack the two L-halves on the partition axis: [N, 128, HALF] — the 2D
    # 128-row DRAM layout loads at fabric rate
    I2 = np.ascontiguousarray(
        I.reshape(N, CIN, 2, HALF).transpose(0, 2, 1, 3).reshape(N, 128, HALF)
    )
    # fold the per-output-channel alpha into the weights and pre-transpose
    # to [K, CIN, COUT] so lhsT tiles are direct SBUF views
    Wt = np.ascontiguousarray(
        (W * alpha.reshape(COUT, 1, 1)).transpose(2, 1, 0).astype(np.float32)
    )
    return [
        {"I": I2[c * NS : (c + 1) * NS], "Wt": Wt, "gamma": gamma, "beta": beta}
        for c in range(NCORES)
    ]


def kernel(I, gamma, beta, W, alpha):
    from concourse.bass_utils import run_bass_kernel_spmd

    nc = _get_nc()
    in_maps = _make_in_maps(I, gamma, beta, W, alpha)
    res = run_bass_kernel_spmd(nc, in_maps, list(range(NCORES)))
    return np.concatenate([res.results[c]["out"] for c in range(NCORES)], axis=0)

